# revision 13
# baseline (speedup 1.0000x reference)
"""Distributed Trainium2 (8-core) kernel for CausalSelfAttention.

Problem: B=2, T=2048, D=2048, NH=16 q-heads, NKV=4 kv-heads, HD=128.
  q,k,v projections -> RMSNorm(q,k) over head dim -> RoPE(q,k) -> q*gain
  -> v += ve_embed -> GQA causal softmax attention -> out proj Wo.

Sharding (8 cores = 2 batch groups x 4 tensor-parallel ranks):
  core (b*4 + c) handles batch b, q-heads [4c,4c+4), kv-head c.
  After attention, per 512-token block j the 4 ranks AllGather their
  yT blocks [512 feat, 512 tok] -> [2048, 512]; each core then computes
  a disjoint 512-column slice of the output projection for those
  tokens, so the host-side unshard is a pure concatenation.

Single fused pipeline: projection tiles, attention blocks, and
out-projection blocks are interleaved in one emission stream so the
tensor engine never idles (keeps the HAM clock-gate warm) and the
per-block AllToAll overlaps compute.

Compute dtype: bf16 matmuls (f32 PSUM accumulate), f32 softmax/norm math.
Softmax runs without max-subtraction: rms-normed q,k bound |score| by
sqrt(HD) ~= 11.3, so exp() cannot overflow fp32/bf16.
"""

import sys

if "/opt/trn_rl_repo" not in sys.path:
    sys.path.insert(0, "/opt/trn_rl_repo")

from contextlib import ExitStack

import ml_dtypes
import numpy as np

import concourse.bass as bass
import concourse.mybir as mybir
import concourse.tile as tile
from concourse import bacc
from concourse.bass_utils import run_bass_kernel_spmd

BF16 = mybir.dt.bfloat16
F32 = mybir.dt.float32
NPBF16 = ml_dtypes.bfloat16

B, T, D = 2, 2048, 2048
NH, NKV, HD = 16, 4, 128
HPC = NH // NKV          # q-heads per core = 4
QF = HPC * HD            # 512 q features per core
ROPE_BASE = 10000.0
EPS = 1.1920929e-07
NT = T // 128            # 16 token tiles
ND = D // 128            # 16 contraction tiles
NB = T // 512            # 4 blocks of 512 tokens
NCORES = 8
GROUPS = [[0, 1, 2, 3], [4, 5, 6, 7]]
AVOFF = (0, 129, 258, 512)   # av column offsets: each 129-wide window
                             # stays inside one 512-f32 PSUM bank


def _emit(tc, ctx):
    nc = tc.nc

    # ---- DRAM I/O ----
    xt_d = nc.dram_tensor("xt", [D, T], BF16, kind="ExternalInput").ap()
    wq_d = nc.dram_tensor("wq", [D, QF], BF16, kind="ExternalInput").ap()
    wkv_d = nc.dram_tensor("wkv", [D, 2 * HD], BF16, kind="ExternalInput").ap()
    ve_d = nc.dram_tensor("ve", [T, HD], BF16, kind="ExternalInput").ap()
    wo_d = nc.dram_tensor("wo", [D, QF], BF16, kind="ExternalInput").ap()
    cs_d = nc.dram_tensor("cs", [T, HD], BF16, kind="ExternalInput").ap()
    qg_d = nc.dram_tensor("qg", [128, HPC], F32, kind="ExternalInput").ap()
    mask_d = nc.dram_tensor("mask", [128, 128], BF16, kind="ExternalInput").ap()
    id_d = nc.dram_tensor("ident", [128, 128], BF16, kind="ExternalInput").ap()
    out_d = nc.dram_tensor("out", [T, QF], F32, kind="ExternalOutput").ap()

    # per-block AllGather buffers: yT block [512 feat, 512 tok] -> [2048, 512]
    ag_in = [nc.dram_tensor(f"ag_in{j}", [QF, 512], BF16) for j in range(NB)]
    ag_out = [nc.dram_tensor(f"ag_out{j}", [D, 512], BF16) for j in range(NB)]

    # ---- PSUM pools (exactly 8 banks; one accumulation group per bank,
    # since a start=True matmul clears has_written for its whole bank) ----
    # pmm: shared rotation for q-proj / kv-proj / out-proj / y-transpose (2 banks)
    pmm = ctx.enter_context(tc.tile_pool(name="pmm", bufs=2, space="PSUM"))
    # patt: score tiles [128,512], double-buffered (2 banks)
    patt = ctx.enter_context(tc.tile_pool(name="patt", bufs=2, space="PSUM"))
    # pav: 4 attention accumulators, one bank each (4 banks)
    pav = ctx.enter_context(tc.tile_pool(name="pav", bufs=1, space="PSUM"))

    # ---- persistent SBUF ----
    persist = ctx.enter_context(tc.tile_pool(name="persist", bufs=1))
    kT = persist.tile([128, T], BF16, tag="kT", name="kT")
    v_sb = [persist.tile([128, HD + 1], BF16, tag=f"v{t}", name=f"v{t}") for t in range(NT)]
    mask_sb = persist.tile([128, 128], BF16, tag="mask", name="mask")
    qg_bc = persist.tile([128, HPC], F32, tag="qgbc", name="qgbc")
    id_sb = persist.tile([128, 128], BF16, tag="ident", name="ident")
    wq_sb = [persist.tile([128, QF], BF16, tag=f"wq{d}", name=f"wq{d}") for d in range(ND)]
    wkv_sb = [persist.tile([128, 2 * HD], BF16, tag=f"wkv{d}", name=f"wkv{d}") for d in range(ND)]
    wo_sb = [persist.tile([128, QF], BF16, tag=f"wo{f}", name=f"wo{f}") for f in range(ND)]
    cs_sb = [persist.tile([128, HD], BF16, tag=f"cs{t}", name=f"cs{t}") for t in range(NT)]
    ve_sb = [persist.tile([128, HD], BF16, tag=f"ve{t}", name=f"ve{t}") for t in range(NT)]

    # rotating pools
    pxt = ctx.enter_context(tc.tile_pool(name="pxt", bufs=2))      # x.T chunks
    pq = ctx.enter_context(tc.tile_pool(name="pq", bufs=2))        # qT per block
    p1s = ctx.enter_context(tc.tile_pool(name="p1s", bufs=3))      # proj scratch
    p2s = ctx.enter_context(tc.tile_pool(name="p2s", bufs=4))      # attn scratch
    pyb = ctx.enter_context(tc.tile_pool(name="pyb", bufs=2))      # ybm tiles
    pyf = ctx.enter_context(tc.tile_pool(name="pyf", bufs=2))      # a2a recv
    pos = ctx.enter_context(tc.tile_pool(name="pos", bufs=2))      # out staging

    # ---- input DMA: gpsimd queue (small consts + cs/ve/wo) ----
    nc.gpsimd.dma_start(id_sb[:], id_d[:, :])
    nc.gpsimd.dma_start(mask_sb[:], mask_d[:, :])
    nc.gpsimd.dma_start(qg_bc[:], qg_d[:, :])
    for t in range(NT):
        nc.gpsimd.dma_start(cs_sb[t][:], cs_d[128 * t : 128 * (t + 1), :])
        nc.gpsimd.dma_start(ve_sb[t][:], ve_d[128 * t : 128 * (t + 1), :])
    for f in range(ND):
        nc.gpsimd.dma_start(wo_sb[f][:], wo_d[128 * f : 128 * (f + 1), :])

    # ---- scalar queue: weights ----
    for d in range(ND):
        nc.scalar.dma_start(wq_sb[d][:], wq_d[128 * d : 128 * (d + 1), :])
    for d in range(ND):
        nc.scalar.dma_start(wkv_sb[d][:], wkv_d[128 * d : 128 * (d + 1), :])

    # ---- sync queue: x.T column chunks (rotating, prefetch depth 2) ----
    xt_aps = {}

    def load_xt_chunk(c):
        aps = []
        for d in range(ND):
            a = pxt.tile([128, 512], BF16, tag=f"xt{d}", name=f"xt{d}c{c}")
            nc.sync.dma_start(a[:], xt_d[128 * d : 128 * (d + 1), 512 * c : 512 * (c + 1)])
            aps.append(a)
        xt_aps[c] = aps

    load_xt_chunk(0)
    load_xt_chunk(1)

    qT_blks = []

    def proj_tile(t):
        c, tc_ = t // 4, t % 4
        if t == 4 and 2 not in xt_aps:
            load_xt_chunk(2)
        if t == 8 and 3 not in xt_aps:
            load_xt_chunk(3)
        if tc_ == 0:
            qb = pq.tile([128, 4 * 512], BF16, tag="qT", name=f"qT{c}")
            qT_blks.append(qb)
        qb = qT_blks[c]
        xt = xt_aps[c]

        q_ps = pmm.tile([128, QF], F32, tag="mm", name="qps")
        for d in range(ND):
            nc.tensor.matmul(
                q_ps[:], xt[d][:, 128 * tc_ : 128 * (tc_ + 1)], wq_sb[d][:],
                start=(d == 0), stop=(d == ND - 1),
            )
        kv_ps = pmm.tile([128, 2 * HD], F32, tag="mm", name="kvps")
        for d in range(ND):
            nc.tensor.matmul(
                kv_ps[:], xt[d][:, 128 * tc_ : 128 * (tc_ + 1)], wkv_sb[d][:],
                start=(d == 0), stop=(d == ND - 1),
            )

        # RMSNorm stats (scalar engine squares w/ accumulator)
        sq = p1s.tile([128, HD], F32, tag="sq")
        ssq = p1s.tile([128, HPC + 1], F32, tag="ssq")
        for h in range(HPC):
            nc.scalar.activation(
                sq[:], q_ps[:, HD * h : HD * (h + 1)],
                mybir.ActivationFunctionType.Square,
                accum_out=ssq[:, h : h + 1],
            )
        nc.scalar.activation(
            sq[:], kv_ps[:, 0:HD],
            mybir.ActivationFunctionType.Square,
            accum_out=ssq[:, HPC : HPC + 1],
        )
        nc.vector.tensor_scalar(
            ssq[:], ssq[:], 1.0 / HD, EPS,
            mybir.AluOpType.mult, mybir.AluOpType.add,
        )
        nc.vector.reciprocal(ssq[:], ssq[:])
        rinv = p1s.tile([128, HPC + 1], F32, tag="rinv")
        nc.scalar.sqrt(rinv[:], ssq[:])  # 1/sqrt(mean+eps)

        qn = p1s.tile([128, QF], BF16, tag="qn")
        kn = p1s.tile([128, HD], BF16, tag="kn")
        for h in range(HPC):
            nc.vector.tensor_scalar(
                qn[:, HD * h : HD * (h + 1)], q_ps[:, HD * h : HD * (h + 1)],
                rinv[:, h : h + 1], qg_bc[:, h : h + 1],
                mybir.AluOpType.mult, mybir.AluOpType.mult,
            )
        nc.vector.tensor_scalar(
            kn[:], kv_ps[:, 0:HD], rinv[:, HPC : HPC + 1], None,
            mybir.AluOpType.mult,
        )

        # RoPE per head (cos/sin [128tok, 64])
        co, si = cs_sb[t][:, 0:64], cs_sb[t][:, 64:128]
        q_ro = p1s.tile([128, QF], BF16, tag="qro")
        k_ro = p1s.tile([128, HD], BF16, tag="kro")
        tma = p1s.tile([128, 64], BF16, tag="ropetma")
        tmb = p1s.tile([128, 64], BF16, tag="ropetmb")
        for h in range(HPC + 1):
            if h < HPC:
                x1, x2 = qn[:, 128 * h : 128 * h + 64], qn[:, 128 * h + 64 : 128 * h + 128]
                r1, r2 = q_ro[:, 128 * h : 128 * h + 64], q_ro[:, 128 * h + 64 : 128 * h + 128]
            else:
                x1, x2 = kn[:, 0:64], kn[:, 64:128]
                r1, r2 = k_ro[:, 0:64], k_ro[:, 64:128]
            nc.vector.tensor_mul(tma[:], x1, co)
            nc.vector.tensor_mul(tmb[:], x2, si)
            nc.vector.tensor_sub(r1, tma[:], tmb[:])
            nc.vector.tensor_mul(tma[:], x1, si)
            nc.vector.tensor_mul(tmb[:], x2, co)
            nc.vector.tensor_add(r2, tma[:], tmb[:])

        qdst = qb[:, 512 * tc_ : 512 * (tc_ + 1)].rearrange("p (h x) -> p h x", h=HPC)
        nc.sync.dma_start_transpose(qdst, q_ro[:])
        nc.sync.dma_start_transpose(kT[:, 128 * t : 128 * (t + 1)], k_ro[:])

        nc.vector.tensor_add(v_sb[t][:, 0:HD], kv_ps[:, HD : 2 * HD], ve_sb[t][:])
        nc.vector.memset(v_sb[t][:, HD : HD + 1], 1.0)

    def att_block(j):
        qT_v = qT_blks[j][:, :].rearrange("p (m x) -> p m x", x=512)
        ntk = 4 * j + 4
        seq = [(h, i) for h in range(HPC) for i in range(ntk)]
        sps = {}

        def emit_scores(idx):
            h_, i_ = seq[idx]
            m_lo = max(0, i_ - 4 * j)
            nc_ = 128 * (4 - m_lo)
            s_ps = patt.tile([128, 512], F32, tag="sps", name="sps")
            nc.tensor.matmul(
                s_ps[:, 0:nc_],
                kT[:, 128 * i_ : 128 * (i_ + 1)],
                qT_v[:, m_lo:4, HD * h_ : HD * (h_ + 1)],
                start=True, stop=True,
            )
            sps[idx] = s_ps

        emit_scores(0)
        av = None
        for idx, (h, i_) in enumerate(seq):
            if idx + 1 < len(seq):
                emit_scores(idx + 1)
            m_lo = max(0, i_ - 4 * j)
            nc_ = 128 * (4 - m_lo)
            s_ps = sps.pop(idx)
            pt = p2s.tile([128, 512], BF16, tag="pt")
            nc.scalar.activation(
                pt[:, 0:nc_], s_ps[:, 0:nc_],
                mybir.ActivationFunctionType.Exp,
            )
            if i_ >= 4 * j:
                nc.vector.tensor_mul(pt[:, 0:128], pt[:, 0:128], mask_sb[:])
            if i_ == 0:
                av = [
                    pav.tile([128, HD + 1], F32, tag=f"av{m}", name=f"av{m}")
                    for m in range(4)
                ]
            for m in range(m_lo, 4):
                nc.tensor.matmul(
                    av[m][:],
                    pt[:, 128 * (m - m_lo) : 128 * (m - m_lo + 1)],
                    v_sb[i_][:],
                    start=(i_ == 0), stop=(i_ == 4 * j + m),
                )
            if i_ == ntk - 1:
                yblk = pyb.tile([128, 512], BF16, tag=f"yblk{h}", name=f"yblk{h}")
                for m in range(4):
                    rs = p2s.tile([128, 1], F32, tag="rs")
                    nc.vector.reciprocal(rs[:], av[m][:, HD : HD + 1])
                    y_tok = p2s.tile([128, HD], BF16, tag="ytok")
                    nc.vector.tensor_scalar(
                        y_tok[:], av[m][:, 0:HD], rs[:], None,
                        mybir.AluOpType.mult,
                    )
                    yt_ps = pmm.tile([128, HD], BF16, tag="mm", name="ytps")
                    nc.tensor.transpose(yt_ps[:], y_tok[:], id_sb[:])
                    nc.vector.tensor_copy(yblk[:, 128 * m : 128 * (m + 1)], yt_ps[:])
                nc.sync.dma_start(ag_in[j].ap()[128 * h : 128 * (h + 1), :], yblk[:])
        nc.gpsimd.collective_compute(
            "AllGather",
            mybir.AluOpType.bypass,
            replica_groups=GROUPS,
            ins=[ag_in[j].ap().opt()],
            outs=[ag_out[j].ap().opt()],
        )

    def oproj_block(j):
        yf = pyf.tile([128, ND * 512], BF16, tag="yf", name=f"yf{j}")
        for f in range(ND):
            nc.gpsimd.dma_start(
                yf[:, 512 * f : 512 * (f + 1)],
                ag_out[j].ap()[128 * f : 128 * (f + 1), :],
            )
        for tt in range(4):
            o_ps = pmm.tile([128, 512], F32, tag="mm", name="ops")
            for f in range(ND):
                nc.tensor.matmul(
                    o_ps[:], yf[:, 512 * f + 128 * tt : 512 * f + 128 * (tt + 1)],
                    wo_sb[f][:],
                    start=(f == 0), stop=(f == ND - 1),
                )
            o_sb = pos.tile([128, 512], F32, tag="osb")
            nc.vector.tensor_copy(o_sb[:], o_ps[:])
            nc.gpsimd.dma_start(
                out_d[128 * (4 * j + tt) : 128 * (4 * j + tt + 1), :], o_sb[:]
            )

    # ---- fused pipeline emission ----
    for t in range(5):
        proj_tile(t)            # t0..t4
    att_block(0)
    for t in range(5, 9):
        proj_tile(t)            # t5..t8
    att_block(1)
    oproj_block(0)
    for t in range(9, 13):
        proj_tile(t)            # t9..t12
    att_block(2)
    oproj_block(1)
    for t in range(13, 16):
        proj_tile(t)            # t13..t15
    oproj_block(2)
    att_block(3)
    oproj_block(3)


_CACHED = None


def _build():
    global _CACHED
    if _CACHED is None:
        nc = bacc.Bacc(
            "TRN2", target_bir_lowering=False, debug=False, num_devices=NCORES
        )
        with tile.TileContext(nc) as tc:
            with ExitStack() as ctx:
                _emit(tc, ctx)
        nc.compile()
        _CACHED = nc
    return _CACHED


def _in_maps(x, ve_embed, Wq, Wk, Wv, Wo, q_gain):
    x = np.asarray(x, np.float32)
    ve_embed = np.asarray(ve_embed, np.float32)
    Wq = np.asarray(Wq, np.float32)
    Wk = np.asarray(Wk, np.float32)
    Wv = np.asarray(Wv, np.float32)
    Wo = np.asarray(Wo, np.float32)
    q_gain = np.asarray(q_gain, np.float32)

    tt = np.arange(T, dtype=np.float32)
    inv_freq = (
        1.0 / (ROPE_BASE ** (np.arange(0, HD, 2, dtype=np.float32) / np.float32(HD)))
    ).astype(np.float32)
    f = np.outer(tt, inv_freq)
    cs = np.concatenate([np.cos(f), np.sin(f)], axis=1).astype(NPBF16)

    p = np.arange(128)[:, None]
    w = np.arange(128)[None, :]
    mask = (w >= p).astype(NPBF16)

    maps = []
    for core in range(NCORES):
        b, c = divmod(core, 4)
        qrows = slice(QF * c, QF * (c + 1))
        krows = slice(HD * c, HD * (c + 1))
        maps.append(
            {
                "xt": np.ascontiguousarray(x[b].T).astype(NPBF16),
                "wq": np.ascontiguousarray(Wq[qrows, :].T).astype(NPBF16),
                "wkv": np.ascontiguousarray(
                    np.concatenate([Wk[krows, :], Wv[krows, :]], axis=0).T
                ).astype(NPBF16),
                "ve": np.ascontiguousarray(ve_embed[b][:, krows]).astype(NPBF16),
                "wo": np.ascontiguousarray(Wo[qrows, :].T).astype(NPBF16),
                "cs": cs,
                "qg": np.broadcast_to(
                    q_gain[None, HPC * c : HPC * (c + 1)]
                    * np.float32(1.0 / np.sqrt(HD)),
                    (128, HPC),
                ).copy(),
                "mask": mask,
                "ident": np.eye(128, dtype=NPBF16),
            }
        )
    return maps


def _assemble(results):
    out = np.empty((B, T, D), np.float32)
    for core in range(NCORES):
        b, c = divmod(core, 4)
        out[b][:, QF * c : QF * (c + 1)] = results[core]["out"]
    return out


def run_traced(**inputs):
    nc = _build()
    maps = _in_maps(**inputs)
    r = run_bass_kernel_spmd(nc, maps, core_ids=list(range(NCORES)), trace=True)
    return _assemble(r.results), r


def kernel(**inputs):
    nc = _build()
    maps = _in_maps(**inputs)
    r = run_bass_kernel_spmd(nc, maps, core_ids=list(range(NCORES)))
    return _assemble(r.results)


# revision 15
# speedup vs baseline: 1.1786x; 1.1786x over previous
"""Distributed Trainium2 (8-core) kernel for CausalSelfAttention.

Problem: B=2, T=2048, D=2048, NH=16 q-heads, NKV=4 kv-heads, HD=128.
  q,k,v projections -> RMSNorm(q,k) over head dim -> RoPE(q,k) -> q*gain
  -> v += ve_embed -> GQA causal softmax attention -> out proj Wo.

Sharding (8 cores = 2 batch groups x 4 tensor-parallel ranks):
  core (b*4 + c) handles batch b, q-heads [4c,4c+4), kv-head c.
  After attention, per 512-token block j the 4 ranks AllGather their
  yT blocks [512 feat, 512 tok] -> [2048, 512]; each core then computes
  a disjoint 512-column slice of the output projection for those
  tokens, so the host-side unshard is a pure concatenation.

Single fused pipeline: projection tiles, attention blocks, and
out-projection blocks are interleaved in one emission stream so the
tensor engine never idles (keeps the HAM clock-gate warm) and the
per-block AllToAll overlaps compute.

Compute dtype: bf16 matmuls (f32 PSUM accumulate), f32 softmax/norm math.
Softmax runs without max-subtraction: rms-normed q,k bound |score| by
sqrt(HD) ~= 11.3, so exp() cannot overflow fp32/bf16.
"""

import sys

if "/opt/trn_rl_repo" not in sys.path:
    sys.path.insert(0, "/opt/trn_rl_repo")

from contextlib import ExitStack

import ml_dtypes
import numpy as np

import concourse.bass as bass
import concourse.mybir as mybir
import concourse.tile as tile
from concourse import bacc
from concourse.bass_utils import run_bass_kernel_spmd

BF16 = mybir.dt.bfloat16
F32 = mybir.dt.float32
NPBF16 = ml_dtypes.bfloat16

B, T, D = 2, 2048, 2048
NH, NKV, HD = 16, 4, 128
HPC = NH // NKV          # q-heads per core = 4
QF = HPC * HD            # 512 q features per core
ROPE_BASE = 10000.0
EPS = 1.1920929e-07
NT = T // 128            # 16 token tiles
ND = D // 128            # 16 contraction tiles
NB = T // 512            # 4 blocks of 512 tokens
NCORES = 8
GROUPS = [[0, 1, 2, 3], [4, 5, 6, 7]]
AVOFF = (0, 129, 258, 512)   # av column offsets: each 129-wide window
                             # stays inside one 512-f32 PSUM bank


def _emit(tc, ctx):
    nc = tc.nc

    # ---- DRAM I/O ----
    xt_d = nc.dram_tensor("xt", [D, T], BF16, kind="ExternalInput").ap()
    wq_d = nc.dram_tensor("wq", [D, QF], BF16, kind="ExternalInput").ap()
    wkv_d = nc.dram_tensor("wkv", [D, 2 * HD], BF16, kind="ExternalInput").ap()
    ve_d = nc.dram_tensor("ve", [T, HD], BF16, kind="ExternalInput").ap()
    wo_d = nc.dram_tensor("wo", [D, QF], BF16, kind="ExternalInput").ap()
    cs_d = nc.dram_tensor("cs", [T, HD], BF16, kind="ExternalInput").ap()
    qg_d = nc.dram_tensor("qg", [128, HPC], F32, kind="ExternalInput").ap()
    mask_d = nc.dram_tensor("mask", [128, 128], BF16, kind="ExternalInput").ap()
    id_d = nc.dram_tensor("ident", [128, 128], BF16, kind="ExternalInput").ap()
    out_d = nc.dram_tensor("out", [T, QF], F32, kind="ExternalOutput").ap()

    # per-block AllGather buffers; block 3 is split in two feature halves
    # so its exchange starts halfway through the block's attention.
    ag_in = [nc.dram_tensor(f"ag_in{j}", [QF, 512], BF16) for j in range(3)]
    ag_out = [nc.dram_tensor(f"ag_out{j}", [D, 512], BF16) for j in range(3)]
    ag3_in = [nc.dram_tensor(f"ag3_in{u}", [QF // 2, 512], BF16) for u in range(2)]
    ag3_out = [nc.dram_tensor(f"ag3_out{u}", [D // 2, 512], BF16) for u in range(2)]

    # ---- PSUM pools (8 banks; one accumulation group per bank) ----
    pmm = ctx.enter_context(tc.tile_pool(name="pmm", bufs=2, space="PSUM"))
    patt = ctx.enter_context(tc.tile_pool(name="patt", bufs=2, space="PSUM"))
    pav = ctx.enter_context(tc.tile_pool(name="pav", bufs=1, space="PSUM"))

    # ---- persistent SBUF ----
    persist = ctx.enter_context(tc.tile_pool(name="persist", bufs=1))
    qT_all = persist.tile([128, NT * 512], BF16, tag="qTall", name="qTall")
    kT = persist.tile([128, T], BF16, tag="kT", name="kT")
    v_sb = [persist.tile([128, HD + 1], BF16, tag=f"v{t}", name=f"v{t}") for t in range(NT)]
    mask_sb = persist.tile([128, 128], BF16, tag="mask", name="mask")
    qg_bc = persist.tile([128, HPC], F32, tag="qgbc", name="qgbc")
    id_sb = persist.tile([128, 128], BF16, tag="ident", name="ident")
    wq_sb = [persist.tile([128, QF], BF16, tag=f"wq{d}", name=f"wq{d}") for d in range(ND)]
    wkv_sb = [persist.tile([128, 2 * HD], BF16, tag=f"wkv{d}", name=f"wkv{d}") for d in range(ND)]
    wo_sb = [persist.tile([128, QF], BF16, tag=f"wo{f}", name=f"wo{f}") for f in range(ND)]
    cs_sb = [persist.tile([128, HD], BF16, tag=f"cs{t}", name=f"cs{t}") for t in range(NT)]
    ve_sb = [persist.tile([128, HD], BF16, tag=f"ve{t}", name=f"ve{t}") for t in range(NT)]
    xt_sb = [persist.tile([128, T], BF16, tag=f"xt{d}", name=f"xt{d}") for d in range(ND)]

    # rotating pools
    p1s = ctx.enter_context(tc.tile_pool(name="p1s", bufs=3))      # proj scratch
    p2s = ctx.enter_context(tc.tile_pool(name="p2s", bufs=4))      # attn scratch
    pyb = ctx.enter_context(tc.tile_pool(name="pyb", bufs=2))      # yblk tiles
    pyf = ctx.enter_context(tc.tile_pool(name="pyf", bufs=2))      # AG recv halves
    pos = ctx.enter_context(tc.tile_pool(name="pos", bufs=2))      # out staging

    # ---- input DMA ----
    # gpsimd queue: consts + cs/ve + wo
    nc.gpsimd.dma_start(id_sb[:], id_d[:, :])
    nc.gpsimd.dma_start(mask_sb[:], mask_d[:, :])
    nc.gpsimd.dma_start(qg_bc[:], qg_d[:, :])
    for t in range(NT):
        nc.gpsimd.dma_start(cs_sb[t][:], cs_d[128 * t : 128 * (t + 1), :])
        nc.gpsimd.dma_start(ve_sb[t][:], ve_d[128 * t : 128 * (t + 1), :])
    for f in range(ND):
        nc.gpsimd.dma_start(wo_sb[f][:], wo_d[128 * f : 128 * (f + 1), :])
    # scalar queue: weights
    for d in range(ND):
        nc.scalar.dma_start(wq_sb[d][:], wq_d[128 * d : 128 * (d + 1), :])
    for d in range(ND):
        nc.scalar.dma_start(wkv_sb[d][:], wkv_d[128 * d : 128 * (d + 1), :])
    # sync queue: x.T, token-chunk-major so early tiles land first
    for c in range(4):
        for d in range(ND):
            nc.sync.dma_start(
                xt_sb[d][:, 512 * c : 512 * (c + 1)],
                xt_d[128 * d : 128 * (d + 1), 512 * c : 512 * (c + 1)],
            )

    # warm the PE HAM clock-gate while input DMAs land
    for w in range(64):
        wps = pmm.tile([128, 128], F32, tag="mm", name="warm")
        nc.tensor.matmul(wps[:], id_sb[:], id_sb[:], start=True, stop=True)

    def proj_tile(t):
        q_ps = pmm.tile([128, QF], F32, tag="mm", name="qps")
        for d in range(ND):
            nc.tensor.matmul(
                q_ps[:], xt_sb[d][:, 128 * t : 128 * (t + 1)], wq_sb[d][:],
                start=(d == 0), stop=(d == ND - 1),
            )
        kv_ps = pmm.tile([128, 2 * HD], F32, tag="mm", name="kvps")
        for d in range(ND):
            nc.tensor.matmul(
                kv_ps[:], xt_sb[d][:, 128 * t : 128 * (t + 1)], wkv_sb[d][:],
                start=(d == 0), stop=(d == ND - 1),
            )

        # RMSNorm stats (scalar engine squares w/ accumulator)
        sq = p1s.tile([128, HD], F32, tag="sq")
        ssq = p1s.tile([128, HPC + 1], F32, tag="ssq")
        for h in range(HPC):
            nc.scalar.activation(
                sq[:], q_ps[:, HD * h : HD * (h + 1)],
                mybir.ActivationFunctionType.Square,
                accum_out=ssq[:, h : h + 1],
            )
        nc.scalar.activation(
            sq[:], kv_ps[:, 0:HD],
            mybir.ActivationFunctionType.Square,
            accum_out=ssq[:, HPC : HPC + 1],
        )
        nc.vector.tensor_scalar(
            ssq[:], ssq[:], 1.0 / HD, EPS,
            mybir.AluOpType.mult, mybir.AluOpType.add,
        )
        nc.vector.reciprocal(ssq[:], ssq[:])
        rinv = p1s.tile([128, HPC + 1], F32, tag="rinv")
        nc.scalar.sqrt(rinv[:], ssq[:])  # 1/sqrt(mean+eps)

        qn = p1s.tile([128, QF], BF16, tag="qn")
        kn = p1s.tile([128, HD], BF16, tag="kn")
        for h in range(HPC):
            nc.vector.tensor_scalar(
                qn[:, HD * h : HD * (h + 1)], q_ps[:, HD * h : HD * (h + 1)],
                rinv[:, h : h + 1], qg_bc[:, h : h + 1],
                mybir.AluOpType.mult, mybir.AluOpType.mult,
            )
        nc.vector.tensor_scalar(
            kn[:], kv_ps[:, 0:HD], rinv[:, HPC : HPC + 1], None,
            mybir.AluOpType.mult,
        )

        # RoPE per head (cos/sin [128tok, 64])
        co, si = cs_sb[t][:, 0:64], cs_sb[t][:, 64:128]
        q_ro = p1s.tile([128, QF], BF16, tag="qro")
        k_ro = p1s.tile([128, HD], BF16, tag="kro")
        tma = p1s.tile([128, 64], BF16, tag="ropetma")
        tmb = p1s.tile([128, 64], BF16, tag="ropetmb")
        for h in range(HPC + 1):
            if h < HPC:
                x1, x2 = qn[:, 128 * h : 128 * h + 64], qn[:, 128 * h + 64 : 128 * h + 128]
                r1, r2 = q_ro[:, 128 * h : 128 * h + 64], q_ro[:, 128 * h + 64 : 128 * h + 128]
            else:
                x1, x2 = kn[:, 0:64], kn[:, 64:128]
                r1, r2 = k_ro[:, 0:64], k_ro[:, 64:128]
            nc.vector.tensor_mul(tma[:], x1, co)
            nc.vector.tensor_mul(tmb[:], x2, si)
            nc.vector.tensor_sub(r1, tma[:], tmb[:])
            nc.vector.tensor_mul(tma[:], x1, si)
            nc.vector.tensor_mul(tmb[:], x2, co)
            nc.vector.tensor_add(r2, tma[:], tmb[:])

        qdst = qT_all[:, 512 * t : 512 * (t + 1)].rearrange("p (h x) -> p h x", h=HPC)
        nc.sync.dma_start_transpose(qdst, q_ro[:])
        nc.sync.dma_start_transpose(kT[:, 128 * t : 128 * (t + 1)], k_ro[:])

        nc.vector.tensor_add(v_sb[t][:, 0:HD], kv_ps[:, HD : 2 * HD], ve_sb[t][:])
        nc.vector.memset(v_sb[t][:, HD : HD + 1], 1.0)

    def att_block(j):
        qT_v = qT_all[:, 512 * 4 * j : 512 * 4 * (j + 1)].rearrange(
            "p (m x) -> p m x", x=512
        )
        ntk = 4 * j + 4
        seq = [(h, i) for h in range(HPC) for i in range(ntk)]
        sps = {}

        def emit_scores(idx):
            h_, i_ = seq[idx]
            m_lo = max(0, i_ - 4 * j)
            nc_ = 128 * (4 - m_lo)
            s_ps = patt.tile([128, 512], F32, tag="sps", name="sps")
            nc.tensor.matmul(
                s_ps[:, 0:nc_],
                kT[:, 128 * i_ : 128 * (i_ + 1)],
                qT_v[:, m_lo:4, HD * h_ : HD * (h_ + 1)],
                start=True, stop=True,
            )
            sps[idx] = s_ps

        emit_scores(0)
        av = None
        for idx, (h, i_) in enumerate(seq):
            if idx + 1 < len(seq):
                emit_scores(idx + 1)
            m_lo = max(0, i_ - 4 * j)
            nc_ = 128 * (4 - m_lo)
            s_ps = sps.pop(idx)
            pt = p2s.tile([128, 512], BF16, tag="pt")
            nc.scalar.activation(
                pt[:, 0:nc_], s_ps[:, 0:nc_],
                mybir.ActivationFunctionType.Exp,
            )
            if i_ >= 4 * j:
                nc.vector.tensor_mul(pt[:, 0:128], pt[:, 0:128], mask_sb[:])
            if i_ == 0:
                av = [
                    pav.tile([128, HD + 1], F32, tag=f"av{m}", name=f"av{m}")
                    for m in range(4)
                ]
            for m in range(m_lo, 4):
                nc.tensor.matmul(
                    av[m][:],
                    pt[:, 128 * (m - m_lo) : 128 * (m - m_lo + 1)],
                    v_sb[i_][:],
                    start=(i_ == 0), stop=(i_ == 4 * j + m),
                )
            if i_ == ntk - 1:
                yblk = pyb.tile([128, 512], BF16, tag=f"yblk{h}", name=f"yblk{h}")
                for m in range(4):
                    rs = p2s.tile([128, 1], F32, tag="rs")
                    nc.vector.reciprocal(rs[:], av[m][:, HD : HD + 1])
                    y_tok = p2s.tile([128, HD], BF16, tag="ytok")
                    nc.vector.tensor_scalar(
                        y_tok[:], av[m][:, 0:HD], rs[:], None,
                        mybir.AluOpType.mult,
                    )
                    yt_ps = pmm.tile([128, HD], BF16, tag="mm", name="ytps")
                    nc.tensor.transpose(yt_ps[:], y_tok[:], id_sb[:])
                    nc.vector.tensor_copy(yblk[:, 128 * m : 128 * (m + 1)], yt_ps[:])
                if j < 3:
                    nc.sync.dma_start(
                        ag_in[j].ap()[128 * h : 128 * (h + 1), :], yblk[:]
                    )
                else:
                    nc.sync.dma_start(
                        ag3_in[h // 2].ap()[128 * (h % 2) : 128 * (h % 2 + 1), :],
                        yblk[:],
                    )
                    if h % 2 == 1:
                        nc.gpsimd.collective_compute(
                            "AllGather",
                            mybir.AluOpType.bypass,
                            replica_groups=GROUPS,
                            ins=[ag3_in[h // 2].ap().opt()],
                            outs=[ag3_out[h // 2].ap().opt()],
                        )
        if j < 3:
            nc.gpsimd.collective_compute(
                "AllGather",
                mybir.AluOpType.bypass,
                replica_groups=GROUPS,
                ins=[ag_in[j].ap().opt()],
                outs=[ag_out[j].ap().opt()],
            )

    def oproj_block(j):
        # halves: (src AP, src row-tile offset, global Wo f-index per slot)
        if j < 3:
            halves = [
                (ag_out[j].ap(), 0, list(range(0, 8))),
                (ag_out[j].ap(), 8, list(range(8, 16))),
            ]
        else:
            halves = [
                (ag3_out[0].ap(), 0, [4 * (s // 2) + (s % 2) for s in range(8)]),
                (ag3_out[1].ap(), 0, [4 * (s // 2) + (s % 2) + 2 for s in range(8)]),
            ]
        yfs = []
        for u, (src, roff, _fmap) in enumerate(halves):
            yf = pyf.tile([128, 8 * 512], BF16, tag=f"yf{u}", name=f"yf{j}_{u}")
            for s in range(8):
                nc.gpsimd.dma_start(
                    yf[:, 512 * s : 512 * (s + 1)],
                    src[128 * (roff + s) : 128 * (roff + s + 1), :],
                )
            yfs.append(yf)
        for tt in range(4):
            o_ps = pmm.tile([128, 512], F32, tag="mm", name="ops")
            nmm = 0
            for yf, (src, roff, fmap) in zip(yfs, halves):
                for s in range(8):
                    nc.tensor.matmul(
                        o_ps[:],
                        yf[:, 512 * s + 128 * tt : 512 * s + 128 * (tt + 1)],
                        wo_sb[fmap[s]][:],
                        start=(nmm == 0), stop=(nmm == ND - 1),
                    )
                    nmm += 1
            o_sb = pos.tile([128, 512], F32, tag="osb")
            nc.vector.tensor_copy(o_sb[:], o_ps[:])
            nc.gpsimd.dma_start(
                out_d[128 * (4 * j + tt) : 128 * (4 * j + tt + 1), :], o_sb[:]
            )

    # ---- emission: dense proj phase, then attention/out-proj pipeline ----
    for t in range(NT):
        proj_tile(t)
    att_block(0)
    att_block(1)
    att_block(2)
    oproj_block(0)
    att_block(3)
    oproj_block(1)
    oproj_block(2)
    oproj_block(3)


_CACHED = None


def _build():
    global _CACHED
    if _CACHED is None:
        nc = bacc.Bacc(
            "TRN2", target_bir_lowering=False, debug=False, num_devices=NCORES
        )
        with tile.TileContext(nc) as tc:
            with ExitStack() as ctx:
                _emit(tc, ctx)
        nc.compile()
        _CACHED = nc
    return _CACHED


def _in_maps(x, ve_embed, Wq, Wk, Wv, Wo, q_gain):
    x = np.asarray(x, np.float32)
    ve_embed = np.asarray(ve_embed, np.float32)
    Wq = np.asarray(Wq, np.float32)
    Wk = np.asarray(Wk, np.float32)
    Wv = np.asarray(Wv, np.float32)
    Wo = np.asarray(Wo, np.float32)
    q_gain = np.asarray(q_gain, np.float32)

    tt = np.arange(T, dtype=np.float32)
    inv_freq = (
        1.0 / (ROPE_BASE ** (np.arange(0, HD, 2, dtype=np.float32) / np.float32(HD)))
    ).astype(np.float32)
    f = np.outer(tt, inv_freq)
    cs = np.concatenate([np.cos(f), np.sin(f)], axis=1).astype(NPBF16)

    p = np.arange(128)[:, None]
    w = np.arange(128)[None, :]
    mask = (w >= p).astype(NPBF16)

    maps = []
    for core in range(NCORES):
        b, c = divmod(core, 4)
        qrows = slice(QF * c, QF * (c + 1))
        krows = slice(HD * c, HD * (c + 1))
        maps.append(
            {
                "xt": np.ascontiguousarray(x[b].T).astype(NPBF16),
                "wq": np.ascontiguousarray(Wq[qrows, :].T).astype(NPBF16),
                "wkv": np.ascontiguousarray(
                    np.concatenate([Wk[krows, :], Wv[krows, :]], axis=0).T
                ).astype(NPBF16),
                "ve": np.ascontiguousarray(ve_embed[b][:, krows]).astype(NPBF16),
                "wo": np.ascontiguousarray(Wo[qrows, :].T).astype(NPBF16),
                "cs": cs,
                "qg": np.broadcast_to(
                    q_gain[None, HPC * c : HPC * (c + 1)]
                    * np.float32(1.0 / np.sqrt(HD)),
                    (128, HPC),
                ).copy(),
                "mask": mask,
                "ident": np.eye(128, dtype=NPBF16),
            }
        )
    return maps


def _assemble(results):
    out = np.empty((B, T, D), np.float32)
    for core in range(NCORES):
        b, c = divmod(core, 4)
        out[b][:, QF * c : QF * (c + 1)] = results[core]["out"]
    return out


def run_traced(**inputs):
    nc = _build()
    maps = _in_maps(**inputs)
    r = run_bass_kernel_spmd(nc, maps, core_ids=list(range(NCORES)), trace=True)
    return _assemble(r.results), r


def kernel(**inputs):
    nc = _build()
    maps = _in_maps(**inputs)
    r = run_bass_kernel_spmd(nc, maps, core_ids=list(range(NCORES)))
    return _assemble(r.results)


# revision 17
# speedup vs baseline: 1.3702x; 1.1625x over previous
"""Distributed Trainium2 (8-core) kernel for CausalSelfAttention.

Problem: B=2, T=2048, D=2048, NH=16 q-heads, NKV=4 kv-heads, HD=128.
  q,k,v projections -> RMSNorm(q,k) over head dim -> RoPE(q,k) -> q*gain
  -> v += ve_embed -> GQA causal softmax attention -> out proj Wo.

Sharding (8 cores = 2 batch groups x 4 tensor-parallel ranks):
  core (b*4 + c) handles batch b, q-heads [4c,4c+4), kv-head c.
  After attention, per 512-token block j the 4 ranks AllGather their
  yT blocks [512 feat, 512 tok] -> [2048, 512]; each core then computes
  a disjoint 512-column slice of the output projection for those
  tokens, so the host-side unshard is a pure concatenation.

Single fused pipeline: projection tiles, attention blocks, and
out-projection blocks are interleaved in one emission stream so the
tensor engine never idles (keeps the HAM clock-gate warm) and the
per-block AllToAll overlaps compute.

Compute dtype: bf16 matmuls (f32 PSUM accumulate), f32 softmax/norm math.
Softmax runs without max-subtraction: rms-normed q,k bound |score| by
sqrt(HD) ~= 11.3, so exp() cannot overflow fp32/bf16.
"""

import sys

if "/opt/trn_rl_repo" not in sys.path:
    sys.path.insert(0, "/opt/trn_rl_repo")

from contextlib import ExitStack

import ml_dtypes
import numpy as np

import concourse.bass as bass
import concourse.mybir as mybir
import concourse.tile as tile
from concourse import bacc
from concourse.bass_utils import run_bass_kernel_spmd

BF16 = mybir.dt.bfloat16
F32 = mybir.dt.float32
NPBF16 = ml_dtypes.bfloat16

B, T, D = 2, 2048, 2048
NH, NKV, HD = 16, 4, 128
HPC = NH // NKV          # q-heads per core = 4
QF = HPC * HD            # 512 q features per core
ROPE_BASE = 10000.0
EPS = 1.1920929e-07
NT = T // 128            # 16 token tiles
ND = D // 128            # 16 contraction tiles
NB = T // 512            # 4 blocks of 512 tokens
NCORES = 8
GROUPS = [[0, 1, 2, 3], [4, 5, 6, 7]]
AVOFF = (0, 129, 258, 512)   # av column offsets: each 129-wide window
                             # stays inside one 512-f32 PSUM bank


def _emit(tc, ctx):
    nc = tc.nc

    # ---- DRAM I/O ----
    xt_d = nc.dram_tensor("xt", [D, T], BF16, kind="ExternalInput").ap()
    wq_d = nc.dram_tensor("wq", [D, QF], BF16, kind="ExternalInput").ap()
    wkv_d = nc.dram_tensor("wkv", [D, 2 * HD], BF16, kind="ExternalInput").ap()
    ve_d = nc.dram_tensor("ve", [T, HD], BF16, kind="ExternalInput").ap()
    wo_d = nc.dram_tensor("wo", [D, QF], BF16, kind="ExternalInput").ap()
    cs_d = nc.dram_tensor("cs", [T, 512], BF16, kind="ExternalInput").ap()
    qg_d = nc.dram_tensor("qg", [128, HPC], F32, kind="ExternalInput").ap()
    mask_d = nc.dram_tensor("mask", [128, 128], BF16, kind="ExternalInput").ap()
    id_d = nc.dram_tensor("ident", [128, 128], BF16, kind="ExternalInput").ap()
    out_d = nc.dram_tensor("out", [T, QF], F32, kind="ExternalOutput").ap()

    # per-block AllGather buffers; block 3 is split in two feature halves
    # so its exchange starts halfway through the block's attention.
    ag_in = [nc.dram_tensor(f"ag_in{j}", [QF, 512], BF16) for j in range(3)]
    ag_out = [nc.dram_tensor(f"ag_out{j}", [D, 512], BF16) for j in range(3)]
    ag3_in = [nc.dram_tensor(f"ag3_in{u}", [QF // 2, 512], BF16) for u in range(2)]
    ag3_out = [nc.dram_tensor(f"ag3_out{u}", [D // 2, 512], BF16) for u in range(2)]

    # ---- PSUM: phase 1 pool (4 banks); attention pools open after ----
    from contextlib import ExitStack as _ES
    p1ps_stack = _ES()
    p1ps = p1ps_stack.enter_context(tc.tile_pool(name="p1ps", bufs=2, space="PSUM"))

    # ---- persistent SBUF ----
    persist = ctx.enter_context(tc.tile_pool(name="persist", bufs=1))
    qT_all = persist.tile([128, NT * 512], BF16, tag="qTall", name="qTall")
    kT = persist.tile([128, T], BF16, tag="kT", name="kT")
    v_sb = [persist.tile([128, HD + 1], BF16, tag=f"v{t}", name=f"v{t}") for t in range(NT)]
    mask_sb = persist.tile([128, 128], BF16, tag="mask", name="mask")
    qg_bc = persist.tile([128, HPC], F32, tag="qgbc", name="qgbc")
    id_sb = persist.tile([128, 128], BF16, tag="ident", name="ident")
    wq_sb = [persist.tile([128, QF], BF16, tag=f"wq{d}", name=f"wq{d}") for d in range(ND)]
    wkv_sb = [persist.tile([128, 2 * HD], BF16, tag=f"wkv{d}", name=f"wkv{d}") for d in range(ND)]
    wo_sb = [persist.tile([128, QF], BF16, tag=f"wo{f}", name=f"wo{f}") for f in range(ND)]
    cs_sb = [persist.tile([128, 512], BF16, tag=f"cs{t}", name=f"cs{t}") for t in range(NT)]
    xt_sb = [persist.tile([128, T], BF16, tag=f"xt{d}", name=f"xt{d}") for d in range(ND)]

    # rotating pools
    p1s = ctx.enter_context(tc.tile_pool(name="p1s", bufs=3))      # proj scratch
    p2s = ctx.enter_context(tc.tile_pool(name="p2s", bufs=3))      # attn scratch
    pyb = ctx.enter_context(tc.tile_pool(name="pyb", bufs=1))      # yblk tiles
    pyf = ctx.enter_context(tc.tile_pool(name="pyf", bufs=2))      # AG recv halves
    pos = ctx.enter_context(tc.tile_pool(name="pos", bufs=2))      # out staging

    # ---- input DMA ----
    # gpsimd queue: consts + cs/ve + wo
    nc.gpsimd.dma_start(id_sb[:], id_d[:, :])
    nc.gpsimd.dma_start(mask_sb[:], mask_d[:, :])
    nc.gpsimd.dma_start(qg_bc[:], qg_d[:, :])
    for t in range(NT):
        nc.gpsimd.dma_start(cs_sb[t][:], cs_d[128 * t : 128 * (t + 1), :])
        nc.gpsimd.dma_start(v_sb[t][:, 0:HD], ve_d[128 * t : 128 * (t + 1), :])
    for f in range(ND):
        nc.gpsimd.dma_start(wo_sb[f][:], wo_d[128 * f : 128 * (f + 1), :])
    # scalar queue: weights
    for d in range(ND):
        nc.scalar.dma_start(wq_sb[d][:], wq_d[128 * d : 128 * (d + 1), :])
    for d in range(ND):
        nc.scalar.dma_start(wkv_sb[d][:], wkv_d[128 * d : 128 * (d + 1), :])
    # sync queue: x.T, token-chunk-major so early tiles land first
    for c in range(4):
        for d in range(ND):
            nc.sync.dma_start(
                xt_sb[d][:, 512 * c : 512 * (c + 1)],
                xt_d[128 * d : 128 * (d + 1), 512 * c : 512 * (c + 1)],
            )

    # warm the PE HAM clock-gate while input DMAs land
    for w in range(64):
        wps = p1ps.tile([128, 128], F32, tag="qps", name="warm")
        nc.tensor.matmul(wps[:], id_sb[:], id_sb[:], start=True, stop=True)

    def proj_tile(t):
        q_ps = p1ps.tile([128, QF], F32, tag="qps", name="qps")
        for d in range(ND):
            nc.tensor.matmul(
                q_ps[:], xt_sb[d][:, 128 * t : 128 * (t + 1)], wq_sb[d][:],
                start=(d == 0), stop=(d == ND - 1),
            )
        kv_ps = p1ps.tile([128, 2 * HD], F32, tag="kvps", name="kvps")
        for d in range(ND):
            nc.tensor.matmul(
                kv_ps[:], xt_sb[d][:, 128 * t : 128 * (t + 1)], wkv_sb[d][:],
                start=(d == 0), stop=(d == ND - 1),
            )

        # RMSNorm stats (scalar engine squares w/ accumulator)
        sq = p1s.tile([128, HD], F32, tag="sq")
        ssq = p1s.tile([128, HPC + 1], F32, tag="ssq")
        for h in range(HPC):
            nc.scalar.activation(
                sq[:], q_ps[:, HD * h : HD * (h + 1)],
                mybir.ActivationFunctionType.Square,
                accum_out=ssq[:, h : h + 1],
            )
        nc.scalar.activation(
            sq[:], kv_ps[:, 0:HD],
            mybir.ActivationFunctionType.Square,
            accum_out=ssq[:, HPC : HPC + 1],
        )
        nc.vector.tensor_scalar(
            ssq[:], ssq[:], 1.0 / HD, EPS,
            mybir.AluOpType.mult, mybir.AluOpType.add,
        )
        nc.vector.reciprocal(ssq[:], ssq[:])
        rinv = p1s.tile([128, HPC + 1], F32, tag="rinv")
        nc.scalar.sqrt(rinv[:], ssq[:])  # 1/sqrt(mean+eps)

        qn = p1s.tile([128, QF], BF16, tag="qn")
        kn = p1s.tile([128, HD], BF16, tag="kn")
        for h in range(HPC):
            nc.vector.tensor_scalar(
                qn[:, HD * h : HD * (h + 1)], q_ps[:, HD * h : HD * (h + 1)],
                rinv[:, h : h + 1], qg_bc[:, h : h + 1],
                mybir.AluOpType.mult, mybir.AluOpType.mult,
            )
        nc.vector.tensor_scalar(
            kn[:], kv_ps[:, 0:HD], rinv[:, HPC : HPC + 1], None,
            mybir.AluOpType.mult,
        )

        # RoPE: all 4 q heads batched via strided views (cs replicated x4)
        co4 = cs_sb[t][:, 0:256].rearrange("p (h x) -> p h x", h=HPC)
        si4 = cs_sb[t][:, 256:512].rearrange("p (h x) -> p h x", h=HPC)
        q_ro = p1s.tile([128, QF], BF16, tag="qro")
        k_ro = p1s.tile([128, HD], BF16, tag="kro")
        tma = p1s.tile([128, 256], BF16, tag="ropetma")
        tmb = p1s.tile([128, 256], BF16, tag="ropetmb")
        qn_v = qn[:, :].rearrange("p (h two x) -> p h two x", h=HPC, two=2)
        qro_v = q_ro[:, :].rearrange("p (h two x) -> p h two x", h=HPC, two=2)
        q1, q2 = qn_v[:, :, 0, :], qn_v[:, :, 1, :]
        tma_v = tma[:, :].rearrange("p (h x) -> p h x", h=HPC)
        tmb_v = tmb[:, :].rearrange("p (h x) -> p h x", h=HPC)
        nc.vector.tensor_mul(tma_v, q1, co4)
        nc.vector.tensor_mul(tmb_v, q2, si4)
        nc.vector.tensor_sub(qro_v[:, :, 0, :], tma_v, tmb_v)
        nc.vector.tensor_mul(tma_v, q1, si4)
        nc.vector.tensor_mul(tmb_v, q2, co4)
        nc.vector.tensor_add(qro_v[:, :, 1, :], tma_v, tmb_v)
        co, si = cs_sb[t][:, 0:64], cs_sb[t][:, 256:320]
        x1, x2 = kn[:, 0:64], kn[:, 64:128]
        nc.vector.tensor_mul(tma[:, 0:64], x1, co)
        nc.vector.tensor_mul(tma[:, 64:128], x2, si)
        nc.vector.tensor_sub(k_ro[:, 0:64], tma[:, 0:64], tma[:, 64:128])
        nc.vector.tensor_mul(tma[:, 0:64], x1, si)
        nc.vector.tensor_mul(tma[:, 64:128], x2, co)
        nc.vector.tensor_add(k_ro[:, 64:128], tma[:, 0:64], tma[:, 64:128])

        qdst = qT_all[:, 512 * t : 512 * (t + 1)].rearrange("p (h x) -> p h x", h=HPC)
        nc.sync.dma_start_transpose(qdst, q_ro[:])
        nc.sync.dma_start_transpose(kT[:, 128 * t : 128 * (t + 1)], k_ro[:])

        nc.vector.tensor_add(v_sb[t][:, 0:HD], kv_ps[:, HD : 2 * HD], v_sb[t][:, 0:HD])
        nc.vector.memset(v_sb[t][:, HD : HD + 1], 1.0)

    def att_block(j):
        qT_v = qT_all[:, 512 * 4 * j : 512 * 4 * (j + 1)].rearrange(
            "p (m x) -> p m x", x=512
        )
        ntk = 4 * j + 4
        seq = [(h, i) for h in range(HPC) for i in range(ntk)]
        sps = {}

        def emit_scores(idx):
            h_, i_ = seq[idx]
            m_lo = max(0, i_ - 4 * j)
            nc_ = 128 * (4 - m_lo)
            s_ps = patt.tile([128, 512], F32, tag="sps", name="sps")
            nc.tensor.matmul(
                s_ps[:, 0:nc_],
                kT[:, 128 * i_ : 128 * (i_ + 1)],
                qT_v[:, m_lo:4, HD * h_ : HD * (h_ + 1)],
                start=True, stop=True,
            )
            sps[idx] = s_ps

        emit_scores(0)
        av = None
        for idx, (h, i_) in enumerate(seq):
            if idx + 1 < len(seq):
                emit_scores(idx + 1)
            m_lo = max(0, i_ - 4 * j)
            nc_ = 128 * (4 - m_lo)
            s_ps = sps.pop(idx)
            pt = p2s.tile([128, 512], BF16, tag="pt")
            nc.scalar.activation(
                pt[:, 0:nc_], s_ps[:, 0:nc_],
                mybir.ActivationFunctionType.Exp,
            )
            if i_ >= 4 * j:
                nc.vector.tensor_mul(pt[:, 0:128], pt[:, 0:128], mask_sb[:])
            if i_ == 0:
                av = [
                    pav.tile([128, HD + 1], F32, tag=f"av{m}", name=f"av{m}")
                    for m in range(4)
                ]
            for m in range(m_lo, 4):
                nc.tensor.matmul(
                    av[m][:],
                    pt[:, 128 * (m - m_lo) : 128 * (m - m_lo + 1)],
                    v_sb[i_][:],
                    start=(i_ == 0), stop=(i_ == 4 * j + m),
                )
            if i_ == ntk - 1:
                yblk = pyb.tile([128, 512], BF16, tag=f"yblk{h}", name=f"yblk{h}")
                for m in range(4):
                    rs = p2s.tile([128, 1], F32, tag="rs")
                    nc.vector.reciprocal(rs[:], av[m][:, HD : HD + 1])
                    y_tok = p2s.tile([128, HD], BF16, tag="ytok")
                    nc.vector.tensor_scalar(
                        y_tok[:], av[m][:, 0:HD], rs[:], None,
                        mybir.AluOpType.mult,
                    )
                    yt_ps = pmm.tile([128, HD], BF16, tag="mm", name="ytps")
                    nc.tensor.transpose(yt_ps[:], y_tok[:], id_sb[:])
                    nc.vector.tensor_copy(yblk[:, 128 * m : 128 * (m + 1)], yt_ps[:])
                if j < 3:
                    nc.sync.dma_start(
                        ag_in[j].ap()[128 * h : 128 * (h + 1), :], yblk[:]
                    )
                else:
                    nc.sync.dma_start(
                        ag3_in[h // 2].ap()[128 * (h % 2) : 128 * (h % 2 + 1), :],
                        yblk[:],
                    )
                    if h % 2 == 1:
                        nc.gpsimd.collective_compute(
                            "AllGather",
                            mybir.AluOpType.bypass,
                            replica_groups=GROUPS,
                            ins=[ag3_in[h // 2].ap().opt()],
                            outs=[ag3_out[h // 2].ap().opt()],
                        )
        if j < 3:
            nc.gpsimd.collective_compute(
                "AllGather",
                mybir.AluOpType.bypass,
                replica_groups=GROUPS,
                ins=[ag_in[j].ap().opt()],
                outs=[ag_out[j].ap().opt()],
            )

    def halves_of(j):
        # (src AP, src row-tile offset, global Wo f-index per slot)
        if j < 3:
            return [
                (ag_out[j].ap(), 0, list(range(0, 8))),
                (ag_out[j].ap(), 8, list(range(8, 16))),
            ]
        return [
            (ag3_out[0].ap(), 0, [4 * (s // 2) + (s % 2) for s in range(8)]),
            (ag3_out[1].ap(), 0, [4 * (s // 2) + (s % 2) + 2 for s in range(8)]),
        ]

    yf_store = {}

    def load_yf(j):
        yfs = []
        for u, (src_, roff, _fmap) in enumerate(halves_of(j)):
            yf = pyf.tile([128, 8 * 512], BF16, tag=f"yf{u}", name=f"yf{j}_{u}")
            for s in range(8):
                nc.gpsimd.dma_start(
                    yf[:, 512 * s : 512 * (s + 1)],
                    src_[128 * (roff + s) : 128 * (roff + s + 1), :],
                )
            yfs.append(yf)
        yf_store[j] = yfs

    def oproj_mm(j):
        yfs = yf_store[j]
        for tt in range(4):
            o_ps = pmm.tile([128, 512], F32, tag="mm", name="ops")
            nmm = 0
            for yf, (src_, roff, fmap) in zip(yfs, halves_of(j)):
                for s in range(8):
                    nc.tensor.matmul(
                        o_ps[:],
                        yf[:, 512 * s + 128 * tt : 512 * s + 128 * (tt + 1)],
                        wo_sb[fmap[s]][:],
                        start=(nmm == 0), stop=(nmm == ND - 1),
                    )
                    nmm += 1
            o_sb = pos.tile([128, 512], F32, tag="osb")
            nc.vector.tensor_copy(o_sb[:], o_ps[:])
            nc.gpsimd.dma_start(
                out_d[128 * (4 * j + tt) : 128 * (4 * j + tt + 1), :], o_sb[:]
            )

    # ---- emission: dense proj phase, then attention/out-proj pipeline ----
    for t in range(NT):
        proj_tile(t)
    p1ps_stack.close()
    patt = ctx.enter_context(tc.tile_pool(name="patt", bufs=2, space="PSUM"))
    pav = ctx.enter_context(tc.tile_pool(name="pav", bufs=1, space="PSUM"))
    pmm = ctx.enter_context(tc.tile_pool(name="pmm", bufs=2, space="PSUM"))
    att_block(0)
    att_block(1)
    load_yf(0)
    att_block(2)
    load_yf(1)
    oproj_mm(0)
    att_block(3)
    load_yf(2)
    oproj_mm(1)
    oproj_mm(2)
    load_yf(3)
    oproj_mm(3)


_CACHED = None


def _build():
    global _CACHED
    if _CACHED is None:
        nc = bacc.Bacc(
            "TRN2", target_bir_lowering=False, debug=False, num_devices=NCORES
        )
        with tile.TileContext(nc) as tc:
            with ExitStack() as ctx:
                _emit(tc, ctx)
        nc.compile()
        _CACHED = nc
    return _CACHED


def _in_maps(x, ve_embed, Wq, Wk, Wv, Wo, q_gain):
    x = np.asarray(x, np.float32)
    ve_embed = np.asarray(ve_embed, np.float32)
    Wq = np.asarray(Wq, np.float32)
    Wk = np.asarray(Wk, np.float32)
    Wv = np.asarray(Wv, np.float32)
    Wo = np.asarray(Wo, np.float32)
    q_gain = np.asarray(q_gain, np.float32)

    tt = np.arange(T, dtype=np.float32)
    inv_freq = (
        1.0 / (ROPE_BASE ** (np.arange(0, HD, 2, dtype=np.float32) / np.float32(HD)))
    ).astype(np.float32)
    f = np.outer(tt, inv_freq)
    cs = np.concatenate(
        [np.tile(np.cos(f), (1, 4)), np.tile(np.sin(f), (1, 4))], axis=1
    ).astype(NPBF16)

    p = np.arange(128)[:, None]
    w = np.arange(128)[None, :]
    mask = (w >= p).astype(NPBF16)

    maps = []
    for core in range(NCORES):
        b, c = divmod(core, 4)
        qrows = slice(QF * c, QF * (c + 1))
        krows = slice(HD * c, HD * (c + 1))
        maps.append(
            {
                "xt": np.ascontiguousarray(x[b].T).astype(NPBF16),
                "wq": np.ascontiguousarray(Wq[qrows, :].T).astype(NPBF16),
                "wkv": np.ascontiguousarray(
                    np.concatenate([Wk[krows, :], Wv[krows, :]], axis=0).T
                ).astype(NPBF16),
                "ve": np.ascontiguousarray(ve_embed[b][:, krows]).astype(NPBF16),
                "wo": np.ascontiguousarray(Wo[qrows, :].T).astype(NPBF16),
                "cs": cs,
                "qg": np.broadcast_to(
                    q_gain[None, HPC * c : HPC * (c + 1)]
                    * np.float32(1.0 / np.sqrt(HD)),
                    (128, HPC),
                ).copy(),
                "mask": mask,
                "ident": np.eye(128, dtype=NPBF16),
            }
        )
    return maps


def _assemble(results):
    out = np.empty((B, T, D), np.float32)
    for core in range(NCORES):
        b, c = divmod(core, 4)
        out[b][:, QF * c : QF * (c + 1)] = results[core]["out"]
    return out


def run_traced(**inputs):
    nc = _build()
    maps = _in_maps(**inputs)
    r = run_bass_kernel_spmd(nc, maps, core_ids=list(range(NCORES)), trace=True)
    return _assemble(r.results), r


def kernel(**inputs):
    nc = _build()
    maps = _in_maps(**inputs)
    r = run_bass_kernel_spmd(nc, maps, core_ids=list(range(NCORES)))
    return _assemble(r.results)


# revision 18
# speedup vs baseline: 1.4118x; 1.0304x over previous
"""Distributed Trainium2 (8-core) kernel for CausalSelfAttention.

Problem: B=2, T=2048, D=2048, NH=16 q-heads, NKV=4 kv-heads, HD=128.
  q,k,v projections -> RMSNorm(q,k) over head dim -> RoPE(q,k) -> q*gain
  -> v += ve_embed -> GQA causal softmax attention -> out proj Wo.

Sharding (8 cores = 2 batch groups x 4 tensor-parallel ranks):
  core (b*4 + c) handles batch b, q-heads [4c,4c+4), kv-head c.
  After attention, per 512-token block j the 4 ranks AllGather their
  yT blocks [512 feat, 512 tok] -> [2048, 512]; each core then computes
  a disjoint 512-column slice of the output projection for those
  tokens, so the host-side unshard is a pure concatenation.

Single fused pipeline: projection tiles, attention blocks, and
out-projection blocks are interleaved in one emission stream so the
tensor engine never idles (keeps the HAM clock-gate warm) and the
per-block AllToAll overlaps compute.

Compute dtype: bf16 matmuls (f32 PSUM accumulate), f32 softmax/norm math.
Softmax runs without max-subtraction: rms-normed q,k bound |score| by
sqrt(HD) ~= 11.3, so exp() cannot overflow fp32/bf16.
"""

import sys

if "/opt/trn_rl_repo" not in sys.path:
    sys.path.insert(0, "/opt/trn_rl_repo")

from contextlib import ExitStack

import ml_dtypes
import numpy as np

import concourse.bass as bass
import concourse.mybir as mybir
import concourse.tile as tile
from concourse import bacc
from concourse.bass_utils import run_bass_kernel_spmd

BF16 = mybir.dt.bfloat16
F32 = mybir.dt.float32
NPBF16 = ml_dtypes.bfloat16

B, T, D = 2, 2048, 2048
NH, NKV, HD = 16, 4, 128
HPC = NH // NKV          # q-heads per core = 4
QF = HPC * HD            # 512 q features per core
ROPE_BASE = 10000.0
EPS = 1.1920929e-07
NT = T // 128            # 16 token tiles
ND = D // 128            # 16 contraction tiles
NB = T // 512            # 4 blocks of 512 tokens
NCORES = 8
GROUPS = [[0, 1, 2, 3], [4, 5, 6, 7]]
AVOFF = (0, 129, 258, 512)   # av column offsets: each 129-wide window
                             # stays inside one 512-f32 PSUM bank


def _emit(tc, ctx):
    nc = tc.nc

    # ---- DRAM I/O ----
    xt_d = nc.dram_tensor("xt", [D, T], BF16, kind="ExternalInput").ap()
    wq_d = nc.dram_tensor("wq", [D, QF], BF16, kind="ExternalInput").ap()
    wkv_d = nc.dram_tensor("wkv", [D, 2 * HD], BF16, kind="ExternalInput").ap()
    ve_d = nc.dram_tensor("ve", [T, HD], BF16, kind="ExternalInput").ap()
    wo_d = nc.dram_tensor("wo", [D, QF], BF16, kind="ExternalInput").ap()
    cs_d = nc.dram_tensor("cs", [T, 512], BF16, kind="ExternalInput").ap()
    qg_d = nc.dram_tensor("qg", [128, HPC], F32, kind="ExternalInput").ap()
    mask_d = nc.dram_tensor("mask", [128, 128], BF16, kind="ExternalInput").ap()
    id_d = nc.dram_tensor("ident", [128, 128], BF16, kind="ExternalInput").ap()
    out_d = nc.dram_tensor("out", [T, QF], F32, kind="ExternalOutput").ap()

    # per-block AllGather buffers; block 3 is split in two feature halves
    # so its exchange starts halfway through the block's attention.
    ag_in = [nc.dram_tensor(f"ag_in{j}", [QF, 512], BF16) for j in range(3)]
    ag_out = [nc.dram_tensor(f"ag_out{j}", [D, 512], BF16) for j in range(3)]
    ag3_in = [nc.dram_tensor(f"ag3_in{u}", [QF // 2, 512], BF16) for u in range(2)]
    ag3_out = [nc.dram_tensor(f"ag3_out{u}", [D // 2, 512], BF16) for u in range(2)]
    dum_in = nc.dram_tensor("dum_in", [128, 8], BF16)
    dum_out = nc.dram_tensor("dum_out", [512, 8], BF16)

    # ---- PSUM: phase 1 pool (4 banks); attention pools open after ----
    from contextlib import ExitStack as _ES
    p1ps_stack = _ES()
    p1ps = p1ps_stack.enter_context(tc.tile_pool(name="p1ps", bufs=2, space="PSUM"))

    # ---- persistent SBUF ----
    persist = ctx.enter_context(tc.tile_pool(name="persist", bufs=1))
    qT_all = persist.tile([128, NT * 512], BF16, tag="qTall", name="qTall")
    kT = persist.tile([128, T], BF16, tag="kT", name="kT")
    v_sb = [persist.tile([128, HD + 1], BF16, tag=f"v{t}", name=f"v{t}") for t in range(NT)]
    mask_sb = persist.tile([128, 128], BF16, tag="mask", name="mask")
    qg_bc = persist.tile([128, HPC], F32, tag="qgbc", name="qgbc")
    id_sb = persist.tile([128, 128], BF16, tag="ident", name="ident")
    wq_sb = [persist.tile([128, QF], BF16, tag=f"wq{d}", name=f"wq{d}") for d in range(ND)]
    wkv_sb = [persist.tile([128, 2 * HD], BF16, tag=f"wkv{d}", name=f"wkv{d}") for d in range(ND)]
    wo_sb = [persist.tile([128, QF], BF16, tag=f"wo{f}", name=f"wo{f}") for f in range(ND)]
    cs_sb = [persist.tile([128, 512], BF16, tag=f"cs{t}", name=f"cs{t}") for t in range(NT)]
    xt_sb = [persist.tile([128, T], BF16, tag=f"xt{d}", name=f"xt{d}") for d in range(ND)]

    # rotating pools
    p1s = ctx.enter_context(tc.tile_pool(name="p1s", bufs=3))      # proj scratch
    p2s = ctx.enter_context(tc.tile_pool(name="p2s", bufs=3))      # attn scratch
    pyb = ctx.enter_context(tc.tile_pool(name="pyb", bufs=1))      # yblk tiles
    pyf = ctx.enter_context(tc.tile_pool(name="pyf", bufs=2))      # AG recv halves
    pos = ctx.enter_context(tc.tile_pool(name="pos", bufs=2))      # out staging

    # ---- input DMA ----
    # dummy collective first: absorbs the first-collective barrier/ring
    # warm-up cost while phase 1 computes
    nc.gpsimd.collective_compute(
        "AllGather", mybir.AluOpType.bypass, replica_groups=GROUPS,
        ins=[dum_in.ap().opt()], outs=[dum_out.ap().opt()],
    )
    # gpsimd queue: consts + cs/ve + wo
    nc.gpsimd.dma_start(id_sb[:], id_d[:, :])
    nc.gpsimd.dma_start(mask_sb[:], mask_d[:, :])
    nc.gpsimd.dma_start(qg_bc[:], qg_d[:, :])
    for t in range(NT):
        nc.gpsimd.dma_start(cs_sb[t][:], cs_d[128 * t : 128 * (t + 1), :])
        nc.gpsimd.dma_start(v_sb[t][:, 0:HD], ve_d[128 * t : 128 * (t + 1), :])
    for f in range(ND):
        nc.gpsimd.dma_start(wo_sb[f][:], wo_d[128 * f : 128 * (f + 1), :])
    # scalar queue: weights
    for d in range(ND):
        nc.scalar.dma_start(wq_sb[d][:], wq_d[128 * d : 128 * (d + 1), :])
    for d in range(ND):
        nc.scalar.dma_start(wkv_sb[d][:], wkv_d[128 * d : 128 * (d + 1), :])
    # sync queue: x.T, token-chunk-major so early tiles land first
    for c in range(4):
        for d in range(ND):
            nc.sync.dma_start(
                xt_sb[d][:, 512 * c : 512 * (c + 1)],
                xt_d[128 * d : 128 * (d + 1), 512 * c : 512 * (c + 1)],
            )

    # warm the PE HAM clock-gate while input DMAs land
    for w in range(52):
        wps = p1ps.tile([128, 512], F32, tag="qps", name="warm")
        nc.tensor.matmul(wps[:], id_sb[:], cs_sb[0][:], start=True, stop=True)

    def proj_tile(t):
        q_ps = p1ps.tile([128, QF], F32, tag="qps", name="qps")
        for d in range(ND):
            nc.tensor.matmul(
                q_ps[:], xt_sb[d][:, 128 * t : 128 * (t + 1)], wq_sb[d][:],
                start=(d == 0), stop=(d == ND - 1),
            )
        kv_ps = p1ps.tile([128, 2 * HD], F32, tag="kvps", name="kvps")
        for d in range(ND):
            nc.tensor.matmul(
                kv_ps[:], xt_sb[d][:, 128 * t : 128 * (t + 1)], wkv_sb[d][:],
                start=(d == 0), stop=(d == ND - 1),
            )

        # RMSNorm stats (scalar engine squares w/ accumulator)
        sq = p1s.tile([128, HD], F32, tag="sq")
        ssq = p1s.tile([128, HPC + 1], F32, tag="ssq")
        for h in range(HPC):
            nc.scalar.activation(
                sq[:], q_ps[:, HD * h : HD * (h + 1)],
                mybir.ActivationFunctionType.Square,
                accum_out=ssq[:, h : h + 1],
            )
        nc.scalar.activation(
            sq[:], kv_ps[:, 0:HD],
            mybir.ActivationFunctionType.Square,
            accum_out=ssq[:, HPC : HPC + 1],
        )
        nc.vector.tensor_scalar(
            ssq[:], ssq[:], 1.0 / HD, EPS,
            mybir.AluOpType.mult, mybir.AluOpType.add,
        )
        nc.vector.reciprocal(ssq[:], ssq[:])
        rinv = p1s.tile([128, HPC + 1], F32, tag="rinv")
        nc.scalar.sqrt(rinv[:], ssq[:])  # 1/sqrt(mean+eps)

        qn = p1s.tile([128, QF], BF16, tag="qn")
        kn = p1s.tile([128, HD], BF16, tag="kn")
        for h in range(HPC):
            nc.vector.tensor_scalar(
                qn[:, HD * h : HD * (h + 1)], q_ps[:, HD * h : HD * (h + 1)],
                rinv[:, h : h + 1], qg_bc[:, h : h + 1],
                mybir.AluOpType.mult, mybir.AluOpType.mult,
            )
        nc.vector.tensor_scalar(
            kn[:], kv_ps[:, 0:HD], rinv[:, HPC : HPC + 1], None,
            mybir.AluOpType.mult,
        )

        # RoPE: all 4 q heads batched via strided views (cs replicated x4)
        co4 = cs_sb[t][:, 0:256].rearrange("p (h x) -> p h x", h=HPC)
        si4 = cs_sb[t][:, 256:512].rearrange("p (h x) -> p h x", h=HPC)
        q_ro = p1s.tile([128, QF], BF16, tag="qro")
        k_ro = p1s.tile([128, HD], BF16, tag="kro")
        tma = p1s.tile([128, 256], BF16, tag="ropetma")
        tmb = p1s.tile([128, 256], BF16, tag="ropetmb")
        qn_v = qn[:, :].rearrange("p (h two x) -> p h two x", h=HPC, two=2)
        qro_v = q_ro[:, :].rearrange("p (h two x) -> p h two x", h=HPC, two=2)
        q1, q2 = qn_v[:, :, 0, :], qn_v[:, :, 1, :]
        tma_v = tma[:, :].rearrange("p (h x) -> p h x", h=HPC)
        tmb_v = tmb[:, :].rearrange("p (h x) -> p h x", h=HPC)
        nc.vector.tensor_mul(tma_v, q1, co4)
        nc.vector.tensor_mul(tmb_v, q2, si4)
        nc.vector.tensor_sub(qro_v[:, :, 0, :], tma_v, tmb_v)
        nc.vector.tensor_mul(tma_v, q1, si4)
        nc.vector.tensor_mul(tmb_v, q2, co4)
        nc.vector.tensor_add(qro_v[:, :, 1, :], tma_v, tmb_v)
        co, si = cs_sb[t][:, 0:64], cs_sb[t][:, 256:320]
        x1, x2 = kn[:, 0:64], kn[:, 64:128]
        nc.vector.tensor_mul(tma[:, 0:64], x1, co)
        nc.vector.tensor_mul(tma[:, 64:128], x2, si)
        nc.vector.tensor_sub(k_ro[:, 0:64], tma[:, 0:64], tma[:, 64:128])
        nc.vector.tensor_mul(tma[:, 0:64], x1, si)
        nc.vector.tensor_mul(tma[:, 64:128], x2, co)
        nc.vector.tensor_add(k_ro[:, 64:128], tma[:, 0:64], tma[:, 64:128])

        qdst = qT_all[:, 512 * t : 512 * (t + 1)].rearrange("p (h x) -> p h x", h=HPC)
        nc.sync.dma_start_transpose(qdst, q_ro[:])
        nc.sync.dma_start_transpose(kT[:, 128 * t : 128 * (t + 1)], k_ro[:])

        nc.vector.tensor_add(v_sb[t][:, 0:HD], kv_ps[:, HD : 2 * HD], v_sb[t][:, 0:HD])
        nc.vector.memset(v_sb[t][:, HD : HD + 1], 1.0)

    def att_block(j):
        qT_v = qT_all[:, 512 * 4 * j : 512 * 4 * (j + 1)].rearrange(
            "p (m x) -> p m x", x=512
        )
        ntk = 4 * j + 4
        seq = [(h, i) for h in range(HPC) for i in range(ntk)]
        sps = {}

        def emit_scores(idx):
            h_, i_ = seq[idx]
            m_lo = max(0, i_ - 4 * j)
            nc_ = 128 * (4 - m_lo)
            s_ps = patt.tile([128, 512], F32, tag="sps", name="sps")
            nc.tensor.matmul(
                s_ps[:, 0:nc_],
                kT[:, 128 * i_ : 128 * (i_ + 1)],
                qT_v[:, m_lo:4, HD * h_ : HD * (h_ + 1)],
                start=True, stop=True,
            )
            sps[idx] = s_ps

        emit_scores(0)
        av = None
        for idx, (h, i_) in enumerate(seq):
            if idx + 1 < len(seq):
                emit_scores(idx + 1)
            m_lo = max(0, i_ - 4 * j)
            nc_ = 128 * (4 - m_lo)
            s_ps = sps.pop(idx)
            pt = p2s.tile([128, 512], BF16, tag="pt")
            nc.scalar.activation(
                pt[:, 0:nc_], s_ps[:, 0:nc_],
                mybir.ActivationFunctionType.Exp,
            )
            if i_ >= 4 * j:
                nc.vector.tensor_mul(pt[:, 0:128], pt[:, 0:128], mask_sb[:])
            if i_ == 0:
                av = [
                    pav.tile([128, HD + 1], F32, tag=f"av{m}", name=f"av{m}")
                    for m in range(4)
                ]
            for m in range(m_lo, 4):
                nc.tensor.matmul(
                    av[m][:],
                    pt[:, 128 * (m - m_lo) : 128 * (m - m_lo + 1)],
                    v_sb[i_][:],
                    start=(i_ == 0), stop=(i_ == 4 * j + m),
                )
            if i_ == ntk - 1:
                yblk = pyb.tile([128, 512], BF16, tag=f"yblk{h}", name=f"yblk{h}")
                for m in range(4):
                    rs = p2s.tile([128, 1], F32, tag="rs")
                    nc.vector.reciprocal(rs[:], av[m][:, HD : HD + 1])
                    y_tok = p2s.tile([128, HD], BF16, tag="ytok")
                    nc.vector.tensor_scalar(
                        y_tok[:], av[m][:, 0:HD], rs[:], None,
                        mybir.AluOpType.mult,
                    )
                    yt_ps = pmm.tile([128, HD], BF16, tag="mm", name="ytps")
                    nc.tensor.transpose(yt_ps[:], y_tok[:], id_sb[:])
                    nc.vector.tensor_copy(yblk[:, 128 * m : 128 * (m + 1)], yt_ps[:])
                if j < 3:
                    nc.sync.dma_start(
                        ag_in[j].ap()[128 * h : 128 * (h + 1), :], yblk[:]
                    )
                else:
                    nc.sync.dma_start(
                        ag3_in[h // 2].ap()[128 * (h % 2) : 128 * (h % 2 + 1), :],
                        yblk[:],
                    )
                    if h % 2 == 1:
                        nc.gpsimd.collective_compute(
                            "AllGather",
                            mybir.AluOpType.bypass,
                            replica_groups=GROUPS,
                            ins=[ag3_in[h // 2].ap().opt()],
                            outs=[ag3_out[h // 2].ap().opt()],
                        )
        if j < 3:
            nc.gpsimd.collective_compute(
                "AllGather",
                mybir.AluOpType.bypass,
                replica_groups=GROUPS,
                ins=[ag_in[j].ap().opt()],
                outs=[ag_out[j].ap().opt()],
            )

    def halves_of(j):
        # (src AP, src row-tile offset, global Wo f-index per slot)
        if j < 3:
            return [
                (ag_out[j].ap(), 0, list(range(0, 8))),
                (ag_out[j].ap(), 8, list(range(8, 16))),
            ]
        return [
            (ag3_out[0].ap(), 0, [4 * (s // 2) + (s % 2) for s in range(8)]),
            (ag3_out[1].ap(), 0, [4 * (s // 2) + (s % 2) + 2 for s in range(8)]),
        ]

    yf_store = {}

    def load_yf(j):
        yfs = []
        for u, (src_, roff, _fmap) in enumerate(halves_of(j)):
            yf = pyf.tile([128, 8 * 512], BF16, tag=f"yf{u}", name=f"yf{j}_{u}")
            sview = src_[128 * roff : 128 * (roff + 8), :].rearrange(
                "(s p) c -> p s c", p=128
            )
            dview = yf[:, :].rearrange("p (s c) -> p s c", c=512)
            nc.gpsimd.dma_start(dview, sview)
            yfs.append(yf)
        yf_store[j] = yfs

    def oproj_mm(j):
        yfs = yf_store[j]
        for tt in range(4):
            o_ps = pmm.tile([128, 512], F32, tag="mm", name="ops")
            nmm = 0
            for yf, (src_, roff, fmap) in zip(yfs, halves_of(j)):
                for s in range(8):
                    nc.tensor.matmul(
                        o_ps[:],
                        yf[:, 512 * s + 128 * tt : 512 * s + 128 * (tt + 1)],
                        wo_sb[fmap[s]][:],
                        start=(nmm == 0), stop=(nmm == ND - 1),
                    )
                    nmm += 1
            o_sb = pos.tile([128, 512], F32, tag="osb")
            nc.vector.tensor_copy(o_sb[:], o_ps[:])
            nc.gpsimd.dma_start(
                out_d[128 * (4 * j + tt) : 128 * (4 * j + tt + 1), :], o_sb[:]
            )

    # ---- emission: dense proj phase, then attention/out-proj pipeline ----
    for t in range(NT):
        proj_tile(t)
    p1ps_stack.close()
    patt = ctx.enter_context(tc.tile_pool(name="patt", bufs=2, space="PSUM"))
    pav = ctx.enter_context(tc.tile_pool(name="pav", bufs=1, space="PSUM"))
    pmm = ctx.enter_context(tc.tile_pool(name="pmm", bufs=2, space="PSUM"))
    att_block(0)
    att_block(1)
    att_block(2)
    load_yf(0)
    oproj_mm(0)
    att_block(3)
    load_yf(1)
    oproj_mm(1)
    load_yf(2)
    oproj_mm(2)
    load_yf(3)
    oproj_mm(3)


_CACHED = None


def _build():
    global _CACHED
    if _CACHED is None:
        nc = bacc.Bacc(
            "TRN2", target_bir_lowering=False, debug=False, num_devices=NCORES
        )
        with tile.TileContext(nc) as tc:
            with ExitStack() as ctx:
                _emit(tc, ctx)
        nc.compile()
        _CACHED = nc
    return _CACHED


def _in_maps(x, ve_embed, Wq, Wk, Wv, Wo, q_gain):
    x = np.asarray(x, np.float32)
    ve_embed = np.asarray(ve_embed, np.float32)
    Wq = np.asarray(Wq, np.float32)
    Wk = np.asarray(Wk, np.float32)
    Wv = np.asarray(Wv, np.float32)
    Wo = np.asarray(Wo, np.float32)
    q_gain = np.asarray(q_gain, np.float32)

    tt = np.arange(T, dtype=np.float32)
    inv_freq = (
        1.0 / (ROPE_BASE ** (np.arange(0, HD, 2, dtype=np.float32) / np.float32(HD)))
    ).astype(np.float32)
    f = np.outer(tt, inv_freq)
    cs = np.concatenate(
        [np.tile(np.cos(f), (1, 4)), np.tile(np.sin(f), (1, 4))], axis=1
    ).astype(NPBF16)

    p = np.arange(128)[:, None]
    w = np.arange(128)[None, :]
    mask = (w >= p).astype(NPBF16)

    maps = []
    for core in range(NCORES):
        b, c = divmod(core, 4)
        qrows = slice(QF * c, QF * (c + 1))
        krows = slice(HD * c, HD * (c + 1))
        maps.append(
            {
                "xt": np.ascontiguousarray(x[b].T).astype(NPBF16),
                "wq": np.ascontiguousarray(Wq[qrows, :].T).astype(NPBF16),
                "wkv": np.ascontiguousarray(
                    np.concatenate([Wk[krows, :], Wv[krows, :]], axis=0).T
                ).astype(NPBF16),
                "ve": np.ascontiguousarray(ve_embed[b][:, krows]).astype(NPBF16),
                "wo": np.ascontiguousarray(Wo[qrows, :].T).astype(NPBF16),
                "cs": cs,
                "qg": np.broadcast_to(
                    q_gain[None, HPC * c : HPC * (c + 1)]
                    * np.float32(1.0 / np.sqrt(HD)),
                    (128, HPC),
                ).copy(),
                "mask": mask,
                "ident": np.eye(128, dtype=NPBF16),
            }
        )
    return maps


def _assemble(results):
    out = np.empty((B, T, D), np.float32)
    for core in range(NCORES):
        b, c = divmod(core, 4)
        out[b][:, QF * c : QF * (c + 1)] = results[core]["out"]
    return out


def run_traced(**inputs):
    nc = _build()
    maps = _in_maps(**inputs)
    r = run_bass_kernel_spmd(nc, maps, core_ids=list(range(NCORES)), trace=True)
    return _assemble(r.results), r


def kernel(**inputs):
    nc = _build()
    maps = _in_maps(**inputs)
    r = run_bass_kernel_spmd(nc, maps, core_ids=list(range(NCORES)))
    return _assemble(r.results)
